# revision 1
# baseline (speedup 1.0000x reference)
"""Trainium2 Bass kernel: single-head causal attention, SPMD over 8 NeuronCores.

Problem: x [4, 2048, 1024] f32; Wq/Wk/Wv [1024, 64]; bq/bk/bv [64].
  q,k,v = x@W + b ; out = softmax(causal(q k^T / 8)) @ v  -> [4, 2048, 64]

Sharding (uniform SPMD structure on every core):
  core c -> batch b = c//2 ; query chunks (cA, cB) = (c%2, 3-c%2), 512 rows
  each (pairing an early with a late chunk balances causal work).  Every core
  computes K/V for its batch's full 2048 rows; collectives would cost more
  than the duplicated projection at this size.

Key layout trick: the k-axis is permuted PER CORE to chunk order
  [cA, 1-cA, 5-cB, cB], so the core's own query columns sit at the STATIC
  positions 0:512 and 1536:2048 of the K/V input -- Q projection needs no
  separate input tensor.  Causality is enforced by data-driven per-partition
  thresholds (thr) against a free-axis iota, which absorb the permutation;
  k-tiles 0..7 for the late slot are causally full for every core and skip
  masking entirely, and the early slot structurally uses only k-tiles 0..7.

  Projections produce Q^T/K^T/V^T [64, rows]; scores are computed transposed
  ([k_part, q_free]) so the attention-weight matrix feeds the AV matmul as
  the moving operand; V is re-transposed through 16 small PE transposes; a
  65th "ones" row on the V tiles makes the AV matmul accumulate the softmax
  denominator for free.  Score matmuls (K=64) are row-packed in pairs into
  disjoint PE row-groups via duplicated K^T/Q^T at partitions 64:127.

dtypes: fp16 SBUF operands (1 cycle/row on the PE; integers exact to 2048
  for the mask iota), fp32 PSUM accumulation, fp32 epilogue + output.
"""

import os
import sys

import numpy as np

if "/opt/trn_rl_repo" not in sys.path:
    sys.path.insert(0, "/opt/trn_rl_repo")

B, S, D, H = 4, 2048, 1024, 64
CH = 512          # query chunk width
QR = 2 * CH       # query rows per core
NKT = S // 128    # 16 k-tiles of 128
SLOT_KT = (8, 16)  # k-tiles consumed by slot A / slot B
SCALE = 1.0 / np.sqrt(H)

_CACHE = {}


def _build_nc():
    import concourse.bacc as bacc
    import concourse.mybir as mybir
    import concourse.tile as tile

    DT = mybir.dt.float16
    F32 = mybir.dt.float32
    Exp = mybir.ActivationFunctionType.Exp
    Copy = mybir.ActivationFunctionType.Copy
    ge = mybir.AluOpType.is_ge
    mult = mybir.AluOpType.mult
    add = mybir.AluOpType.add

    nc = bacc.Bacc("TRN2", target_bir_lowering=False, debug=False, num_devices=8)

    # xk: k-permuted x^T in 16 contiguous [128, 1024] chunks;
    # row block kt*2+h holds dmodel-tile kt, k-position half h.
    xk = nc.dram_tensor("xk", [16 * 128, 1024], DT, kind="ExternalInput")
    wkv = nc.dram_tensor("wkv", [8 * 128, 128], DT, kind="ExternalInput")
    wq = nc.dram_tensor("wq", [128, 8 * H], DT, kind="ExternalInput")
    bkv = nc.dram_tensor("bkv", [128, 1], F32, kind="ExternalInput")
    bq = nc.dram_tensor("bq", [H, 1], F32, kind="ExternalInput")
    qio = nc.dram_tensor("qio", [128, CH], DT, kind="ExternalInput")
    thr = nc.dram_tensor("thr", [128, 2 * NKT], F32, kind="ExternalInput")
    thrb = nc.dram_tensor("thrb", [128, 2 * NKT], F32, kind="ExternalInput")
    idv = nc.dram_tensor("idv", [128, H], DT, kind="ExternalInput")
    id16 = nc.dram_tensor("id16", [H + 1, H + 1], DT, kind="ExternalInput")
    out = nc.dram_tensor("out", [QR, H], DT, kind="ExternalOutput")

    with tile.TileContext(nc) as tc:
        with (
            tc.tile_pool(name="const", bufs=1) as cp,
            tc.tile_pool(name="work", bufs=8) as wp,
            tc.tile_pool(name="epi", bufs=4) as ep,
        ):
            # ---- head: the first matmul needs only wkv[0] + xk[0][0], so
            # those are the very first issues on their engines.
            issue4 = [nc.sync, nc.scalar, nc.gpsimd]
            wkv_sb = cp.tile([128, 8 * 128], DT, tag="wkv", name="wkv")
            xk_sb = [[None, None] for _ in range(8)]

            def _xk_tile(kt, h):
                t = cp.tile([128, 1024], DT, tag=f"xk{kt}_{h}",
                            name=f"xk{kt}_{h}")
                xk_sb[kt][h] = t
                return t, (kt * 2 + h) * 128

            rr = [0]

            def _issue(dst, src, engs=issue4):
                engs[rr[0] % len(engs)].dma_start(dst, src)
                rr[0] += 1

            def _xk_issue(kt, h, nsplit, engs=issue4):
                t, row = _xk_tile(kt, h)
                step = 128 // nsplit
                for s in range(nsplit):
                    _issue(t[s * step:(s + 1) * step, :],
                           xk[row + s * step:row + (s + 1) * step, :], engs)

            # strict consumption order: the PE eats one xk tile every
            # ~1.3us; each tile's chunks (and its weight tile) must be in
            # flight early enough (~16.6GB/s per dma_start stream)
            t0, row0 = _xk_tile(0, 0)
            nc.sync.dma_start(t0[0:64, :], xk[row0:row0 + 64, :])
            nc.scalar.dma_start(t0[64:128, :], xk[row0 + 64:row0 + 128, :])
            nc.gpsimd.dma_start(wkv_sb[:, 0:128], wkv[0:128, :])
            wq_sb = cp.tile([128, 8 * H], DT, tag="wq", name="wq")
            for s in range(4):
                _issue(wq_sb[s * 32:(s + 1) * 32, :],
                       wq[s * 32:(s + 1) * 32, :])
            bkv_sb = cp.tile([128, 1], F32, tag="bkv", name="bkv")
            bq_sb = cp.tile([H, 1], F32, tag="bq", name="bq")
            for kt in range(1, 8):
                _xk_issue(kt, 0, 4 if kt <= 4 else 2)
                _issue(wkv_sb[:, kt * 128:(kt + 1) * 128],
                       wkv[kt * 128:(kt + 1) * 128, :])
                if kt == 2:
                    _issue(bkv_sb[:], bkv[:])
                    _issue(bq_sb[:], bq[:])
            for kt in range(8):
                _xk_issue(kt, 1, 2, engs=[nc.sync, nc.gpsimd])

            # late-use constants (mask iota/thr, identities)
            qio_sb = cp.tile([128, CH], DT, tag="qio", name="qio")
            nc.gpsimd.dma_start(qio_sb[:], qio[:])
            thr_sb = cp.tile([128, 2 * NKT], F32, tag="thr", name="thr")
            nc.gpsimd.dma_start(thr_sb[:], thr[:])
            thrb_sb = cp.tile([128, 2 * NKT], F32, tag="thrb", name="thrb")
            nc.gpsimd.dma_start(thrb_sb[:], thrb[:])
            idv_sb = cp.tile([128, H], DT, tag="idv", name="idv")
            nc.scalar.dma_start(idv_sb[:], idv[:])
            id16_sb = cp.tile([H + 1, H + 1], DT, tag="id16", name="id16")
            nc.gpsimd.dma_start(id16_sb[:], id16[:])

            kvT_sb = cp.tile([128, S], DT, tag="kvT", name="kvT")  # 0:64 K^T, 64:128 V^T
            qT_sb = cp.tile([H, QR], DT, tag="qT", name="qT")      # A cols 0:512, B 512:1024
            v_sb = cp.tile([128, NKT * (H + 1)], DT, tag="v", name="v")
            # duplicates at partitions 64:127 for row-packed score pairs
            ktd_sb = cp.tile([128, S], DT, tag="ktd", name="ktd")
            qTd_sb = cp.tile([128, QR], DT, tag="qTd", name="qTd")
            vtd_sb = cp.tile([64, S], DT, tag="vtd", name="vtd")

            nc.vector.memset(v_sb[:], 1.0)

            # V^T -> V tile transposes, row-packed; pr 0..3 (k-tiles 0..7)
            # are emitted right after h0 to fill PE gaps while h1 streams in
            def v_transpose(pr):
                k0, k1 = 2 * pr, 2 * pr + 1
                t0 = sp.tile([128, H], DT, tag="score", name="vtr0")
                nc.tensor.transpose(
                    t0[:], vtd_sb[:, k0 * 128:(k0 + 1) * 128],
                    idv_sb[0:H, :], tile_position=(0, 0))
                t1 = sp.tile([128, H], DT, tag="score", name="vtr1")
                nc.tensor.transpose(
                    t1[:], kvT_sb[64:128, k1 * 128:(k1 + 1) * 128],
                    idv_sb[64:64 + H, :], tile_position=(64, 0))
                nc.vector.tensor_copy(
                    v_sb[:, k0 * (H + 1):k0 * (H + 1) + H], t0[:])
                nc.vector.tensor_copy(
                    v_sb[:, k1 * (H + 1):k1 * (H + 1) + H], t1[:])

            # ---- projections in two 3-bank PSUM phase scopes so the score
            # pool can allocate after phase h0 and slot-A attention overlaps
            # phase h1 (banks: h1 3 + score 4 = 7; then score 4 + av 4 = 8)
            sp = None
            for h in range(2):
                with tc.tile_pool(name=f"proj_ps{h}", bufs=1,
                                  space="PSUM") as pp:
                    kv_ps = [pp.tile([128, 512], F32, tag=f"kvps{h}{s}",
                                     name=f"kvps{h}{s}") for s in range(2)]
                    q_ps = pp.tile([H, 512], F32, tag=f"qps{h}",
                                   name=f"qps{h}")
                    # q columns: slot A = positions 0:512 (in half 0),
                    # slot B = positions 1536:2048 (in half 1)
                    qcol = slice(0, 512) if h == 0 else slice(512, 1024)
                    for kt in range(8):
                        for sub in range(2):
                            nc.tensor.matmul(
                                kv_ps[sub][:],
                                wkv_sb[:, kt * 128:(kt + 1) * 128],
                                xk_sb[kt][h][:, sub * 512:(sub + 1) * 512],
                                start=(kt == 0), stop=(kt == 7),
                            )
                        nc.tensor.matmul(
                            q_ps[:],
                            wq_sb[:, kt * H:(kt + 1) * H],
                            xk_sb[kt][h][:, qcol],
                            start=(kt == 0), stop=(kt == 7),
                        )
                    for sub in range(2):
                        nb = 2 * h + sub
                        nc.vector.tensor_scalar(
                            kvT_sb[:, nb * 512:(nb + 1) * 512], kv_ps[sub][:],
                            bkv_sb[:], None, add)
                        # K^T duplicate rows 64:127 (small SBUF->SBUF DMA,
                        # off the critical path)
                        nc.sync.dma_start(
                            ktd_sb[H:128, nb * 512:(nb + 1) * 512],
                            kvT_sb[0:H, nb * 512:(nb + 1) * 512])
                        nc.vector.tensor_scalar(
                            vtd_sb[:, nb * 512:(nb + 1) * 512],
                            kv_ps[sub][H:128, :], bkv_sb[H:128, :], None, add)
                    nc.vector.tensor_scalar(
                        qT_sb[:, h * 512:(h + 1) * 512], q_ps[:],
                        bq_sb[:], None, add)
                    nc.scalar.dma_start(
                        qTd_sb[H:128, h * 512:(h + 1) * 512],
                        qT_sb[:, h * 512:(h + 1) * 512])
                if h == 0:
                    sp = tc.alloc_tile_pool(name="score_ps", bufs=4,
                                            space="PSUM")
                    for pr in range(4):
                        v_transpose(pr)
            avpA = tc.alloc_tile_pool(name="avA_ps", bufs=1, space="PSUM")
            avpB = tc.alloc_tile_pool(name="avB_ps", bufs=1, space="PSUM")


            # ---- attention per slot (score pairs row-packed) ----
            for slot in range(2):
                nkt = SLOT_KT[slot]
                avp = avpA if slot == 0 else avpB
                av_e = avp.tile([H + 1, 512], F32, tag=f"avE{slot}",
                                name="avE")
                av_o = avp.tile([H + 1, 512], F32, tag=f"avO{slot}",
                                name="avO")
                # slot B: unmasked tiles (0..7) first -- their V tiles
                # exist already; v8..15 transposes are emitted here and
                # overlap the first pairs' score/exp work
                if slot == 1:
                    # end on a bias-masked pair (8..11): its exp->AV tail
                    # chain skips the diagonal pairs' extra mask multiply
                    kts = list(range(8)) + [12, 13, 14, 15, 8, 9, 10, 11]
                    for pr in range(4, NKT // 2):
                        v_transpose(pr)
                else:
                    kts = list(range(nkt))
                for ki in range(0, nkt, 2):
                    kt0, kt1 = kts[ki], kts[ki + 1]
                    s0 = sp.tile([128, 512], F32, tag="score", name="score0")
                    nc.tensor.matmul(
                        s0[:],
                        kvT_sb[0:H, kt0 * 128:(kt0 + 1) * 128],
                        qT_sb[:, slot * 512:(slot + 1) * 512],
                        start=True, stop=True, tile_position=(0, 0),
                    )
                    s1 = sp.tile([128, 512], F32, tag="score", name="score1")
                    nc.tensor.matmul(
                        s1[:],
                        ktd_sb[H:128, kt1 * 128:(kt1 + 1) * 128],
                        qTd_sb[H:128, slot * 512:(slot + 1) * 512],
                        start=True, stop=True, tile_position=(64, 0),
                    )
                    w_pair = []
                    for kt, s_ps in zip((kt0, kt1), (s0, s1)):
                        idx = slot * NKT + kt
                        w_sb = wp.tile([128, 512], DT, tag="wexp", name="wexp")
                        # bias kills constant fully-masked tiles for free
                        nc.scalar.activation(w_sb[:], s_ps[:], Exp,
                                             bias=thrb_sb[:, idx:idx + 1],
                                             scale=float(SCALE))
                        w_pair.append(w_sb)
                    wav_pair = []
                    for kt, w_sb in zip((kt0, kt1), w_pair):
                        # only diagonal tiles still need an elementwise mask
                        diag = (slot == 0 and kt < 4) or (slot == 1 and kt >= 12)
                        if not diag:
                            wav_pair.append(w_sb)
                            continue
                        idx = slot * NKT + kt
                        m_sb = wp.tile([128, 512], DT, tag="msk", name="msk")
                        nc.vector.tensor_scalar(
                            m_sb[:], qio_sb[:], thr_sb[:, idx:idx + 1], None, ge)
                        wm_sb = wp.tile([128, 512], DT, tag="wm", name="wm")
                        nc.vector.tensor_tensor(
                            wm_sb[:], w_sb[:], m_sb[:], mult)
                        wav_pair.append(wm_sb)
                    for j, (kt, w_av) in enumerate(zip((kt0, kt1), wav_pair)):
                        vs = slice(kt * (H + 1), (kt + 1) * (H + 1))
                        nc.tensor.matmul(
                            av_e[:], v_sb[0:H, vs], w_av[0:H, :],
                            start=(ki + j == 0), stop=(ki + j == nkt - 1),
                            tile_position=(0, 0),
                        )
                        nc.tensor.matmul(
                            av_o[:], v_sb[H:128, vs], w_av[H:128, :],
                            start=(ki + j == 0), stop=(ki + j == nkt - 1),
                            tile_position=(64, 0),
                        )
                # epilogue: sum AV halves (ACT copy + DVE add, fp16),
                # transpose to [128, 65], normalize in f32
                oav_sb = ep.tile([H + 1, 512], DT, tag="oav16", name="oav")
                oc_sb = ep.tile([H + 1, 512], F32, tag="oav", name="oavc")
                for j in range(4):
                    js = slice(j * 128, (j + 1) * 128)
                    nc.scalar.activation(oc_sb[:, js], av_e[:, js], Copy)
                    nc.vector.tensor_tensor(
                        oav_sb[:, js], oc_sb[:, js], av_o[:, js], add)
                for j in range(4):
                    tr_ps = sp.tile([128, H + 1], DT, tag="score", name="otr")
                    nc.tensor.transpose(
                        tr_ps[:],
                        oav_sb[:, j * 128:(j + 1) * 128],
                        id16_sb[0:H + 1, 0:H + 1],
                    )
                    r_sb = ep.tile([128, 1], F32, tag="recip", name="recip")
                    nc.vector.reciprocal(r_sb[:], tr_ps[:, H:H + 1])
                    o_sb = ep.tile([128, H], DT, tag="osb", name="osb")
                    nc.vector.tensor_scalar_mul(o_sb[:], tr_ps[:, 0:H], r_sb[:])
                    row = slot * CH + j * 128
                    # sync/scalar only: a gpsimd-issued store would hold up
                    # gpsimd's end-of-kernel queue drain by ~3us
                    (nc.sync if j % 2 == 0 else nc.scalar).dma_start(
                        out[row:row + 128, :], o_sb[:])

            for pool in (avpB, avpA, sp):
                pool.release()

    nc.compile()
    return nc


def _host_inputs(x, Wq, bq, Wk, bk, Wv, bv):
    """Build the 8 per-core input maps (all SBUF-layout, fp16/f32)."""
    f16 = np.float16
    Wkv = np.concatenate([Wk, Wv], axis=1)          # [D, 128]
    wkv_np = np.ascontiguousarray(Wkv).astype(f16).reshape(8 * 128, 128)
    wq_np = np.zeros((128, 8 * H), dtype=f16)
    for kt in range(8):
        wq_np[:, kt * H:(kt + 1) * H] = Wq[kt * 128:(kt + 1) * 128, :]
    bkv_np = np.concatenate([bk, bv]).reshape(128, 1).astype(np.float32)
    bq_np = bq.reshape(H, 1).astype(np.float32)
    qio_np = np.broadcast_to(np.arange(CH, dtype=f16), (128, CH)).copy()
    idv_np = np.concatenate([np.eye(H), np.eye(H)], axis=0).astype(f16)
    id16_np = np.eye(H + 1, dtype=f16)

    in_maps = []
    for c in range(8):
        b = c // 2
        cA, cB = c % 2, 3 - c % 2
        perm = (cA, 1 - cA, 5 - cB, cB)        # chunk order along k
        xTp = np.concatenate(
            [x[b, p * CH:(p + 1) * CH].T for p in perm], axis=1)  # [D, S]
        xTp = xTp.astype(f16)
        xk_np = np.zeros((16 * 128, 1024), dtype=f16)
        for kt in range(8):
            for h in range(2):
                xk_np[(kt * 2 + h) * 128:(kt * 2 + h + 1) * 128] = \
                    xTp[kt * 128:(kt + 1) * 128, h * 1024:(h + 1) * 1024]
        # k_global of permuted position p: perm[p//512]*512 + p%512
        pos = np.arange(S)
        kg = np.array(perm)[pos // CH] * CH + pos % CH
        thr_np = np.zeros((128, 2 * NKT), dtype=np.float32)
        p = np.arange(128)
        for slot, ck in enumerate((cA, cB)):
            for kt in range(NKT):
                thr_np[:, slot * NKT + kt] = kg[kt * 128 + p] - ck * CH
        thrb_np = np.zeros((128, 2 * NKT), dtype=np.float32)
        for slot in range(2):
            for kt in range(NKT):
                diag = (slot == 0 and kt < 4) or (slot == 1 and kt >= 12)
                if diag:
                    continue
                col = thr_np[:, slot * NKT + kt]
                if np.all(col <= 0):
                    continue          # fully visible -> bias 0
                thrb_np[:, slot * NKT + kt] = -1e5   # fully masked
        in_maps.append({
            "xk": xk_np, "wkv": wkv_np, "wq": wq_np,
            "bkv": bkv_np, "bq": bq_np, "qio": qio_np, "thr": thr_np,
            "thrb": thrb_np, "idv": idv_np, "id16": id16_np,
        })
    return in_maps


def _gather(results, dtype):
    y = np.zeros((B, S, H), dtype=dtype)
    for c in range(8):
        b = c // 2
        cA, cB = c % 2, 3 - c % 2
        o = results[c]["out"]
        y[b, cA * CH:(cA + 1) * CH] = o[:CH]
        y[b, cB * CH:(cB + 1) * CH] = o[CH:]
    return y


def get_nc():
    if "nc" not in _CACHE:
        _CACHE["nc"] = _build_nc()
    return _CACHE["nc"]


def kernel(x, Wq, bq, Wk, bk, Wv, bv, _trace=False, _trace_kwargs=None):
    from concourse.bass_utils import run_bass_kernel_spmd

    x = np.asarray(x, dtype=np.float32)
    Wq, bq = np.asarray(Wq, np.float32), np.asarray(bq, np.float32)
    Wk, bk = np.asarray(Wk, np.float32), np.asarray(bk, np.float32)
    Wv, bv = np.asarray(Wv, np.float32), np.asarray(bv, np.float32)

    nc = get_nc()
    in_maps = _host_inputs(x, Wq, bq, Wk, bk, Wv, bv)
    res = run_bass_kernel_spmd(
        nc, in_maps, core_ids=list(range(8)),
        trace=_trace, **(_trace_kwargs or {}))
    _CACHE["last_result"] = res
    return _gather(res.results, x.dtype)



# revision 2
# speedup vs baseline: 1.0708x; 1.0708x over previous
"""Trainium2 Bass kernel: single-head causal attention, SPMD over 8 NeuronCores.

Problem: x [4, 2048, 1024] f32; Wq/Wk/Wv [1024, 64]; bq/bk/bv [64].
  q,k,v = x@W + b ; out = softmax(causal(q k^T / 8)) @ v  -> [4, 2048, 64]

Sharding (uniform SPMD structure on every core):
  core c -> batch b = c//2 ; query chunks (cA, cB) = (c%2, 3-c%2), 512 rows
  each (pairing an early with a late chunk balances causal work).  Every core
  computes K/V for its batch's full 2048 rows; collectives would cost more
  than the duplicated projection at this size.

Key layout trick: the k-axis is permuted PER CORE to chunk order
  [cA, 1-cA, 5-cB, cB], so the core's own query columns sit at the STATIC
  positions 0:512 and 1536:2048 of the K/V input -- Q projection needs no
  separate input tensor.  Causality is enforced by data-driven per-partition
  thresholds (thr) against a free-axis iota, which absorb the permutation;
  k-tiles 0..7 for the late slot are causally full for every core and skip
  masking entirely, and the early slot structurally uses only k-tiles 0..7.

  Projections produce Q^T/K^T/V^T [64, rows]; scores are computed transposed
  ([k_part, q_free]) so the attention-weight matrix feeds the AV matmul as
  the moving operand; V is re-transposed through 16 small PE transposes; a
  65th "ones" row on the V tiles makes the AV matmul accumulate the softmax
  denominator for free.  Score matmuls (K=64) are row-packed in pairs into
  disjoint PE row-groups via duplicated K^T/Q^T at partitions 64:127.

DMA strategy: ONE dma_start is split across all 16 SDMA engines (~340 GB/s
  at >=512KB), so all inputs are host-packed into three partition-major DRAM
  tensors: xk [128, 16K] fp16 streamed as 8 x 512KB chunks on the sync
  (HWDGE) ring, cst16 (weights/iota/identities) in one DMA on scalar, cst32
  (biases/thresholds) in one DMA on gpsimd.  Output accumulates in SBUF and
  stores once per slot.  This keeps the per-instruction ~0.6us sequencer
  issue cost off the critical path (the old many-small-DMA scheme was
  sequencer-bound at ~67us).

dtypes: fp16 SBUF operands (1 cycle/row on the PE; integers exact to 2048
  for the mask iota), fp32 PSUM accumulation, fp32 epilogue + output.
"""

import os
import sys

import numpy as np

if "/opt/trn_rl_repo" not in sys.path:
    sys.path.insert(0, "/opt/trn_rl_repo")

B, S, D, H = 4, 2048, 1024, 64
CH = 512          # query chunk width
QR = 2 * CH       # query rows per core
NKT = S // 128    # 16 k-tiles of 128
SLOT_KT = (8, 16)  # k-tiles consumed by slot A / slot B
SCALE = 1.0 / np.sqrt(H)

# cst16 column layout
C_WKV = 0          # [128, 1024] 8 kt blocks of [128, 128]
C_WQ = 1024        # [128, 512]  8 kt blocks of [128, 64]
C_QIO = 1536       # [128, 512]  iota 0..511 broadcast
C_IDV = 2048       # [128, 64]   eye(64) stacked twice
C_ID16 = 2112      # [65, 65]    eye(65)
C16_N = 2240       # padded total

# cst32 column layout (f32)
C_BKV = 0          # [128, 1]
C_BQ = 1           # [64, 1]
C_THR = 2          # [128, 32]
C_THRB = 34        # [128, 32]
C32_N = 72

_CACHE = {}


def _build_nc():
    import concourse.bacc as bacc
    import concourse.mybir as mybir
    import concourse.tile as tile

    DT = mybir.dt.float16
    F32 = mybir.dt.float32
    Exp = mybir.ActivationFunctionType.Exp
    Copy = mybir.ActivationFunctionType.Copy
    ge = mybir.AluOpType.is_ge
    mult = mybir.AluOpType.mult
    add = mybir.AluOpType.add

    nc = bacc.Bacc("TRN2", target_bir_lowering=False, debug=False, num_devices=8)

    # xk: k-permuted x^T, partition-major: col block (h*8+kt)*1024 holds
    # dmodel-tile kt, k-position half h.
    xk = nc.dram_tensor("xk", [128, 16 * 1024], DT, kind="ExternalInput")
    cst16 = nc.dram_tensor("cst16", [128, C16_N], DT, kind="ExternalInput")
    cst32 = nc.dram_tensor("cst32", [128, C32_N], F32, kind="ExternalInput")
    out = nc.dram_tensor("out", [128, 8 * H], DT, kind="ExternalOutput")

    with tile.TileContext(nc) as tc:
        with (
            tc.tile_pool(name="const", bufs=1) as cp,
            tc.tile_pool(name="work", bufs=8) as wp,
            tc.tile_pool(name="epi", bufs=4) as ep,
        ):
            # ---- head: three big input streams; each dma_start fans out
            # over all 16 SDMA engines, so few+large beats many+small.
            cst16_sb = cp.tile([128, C16_N], DT, tag="cst16", name="cst16")
            nc.scalar.dma_start(cst16_sb[:], cst16[:])
            cst32_sb = cp.tile([128, C32_N], F32, tag="cst32", name="cst32")
            nc.gpsimd.dma_start(cst32_sb[:], cst32[:])
            xk_sb = cp.tile([128, 16 * 1024], DT, tag="xk", name="xk")
            for c in range(8):
                nc.sync.dma_start(xk_sb[:, c * 2048:(c + 1) * 2048],
                                  xk[:, c * 2048:(c + 1) * 2048])

            wkv_sb = cst16_sb[:, C_WKV:C_WKV + 1024]
            wq_sb = cst16_sb[:, C_WQ:C_WQ + 512]
            qio_sb = cst16_sb[:, C_QIO:C_QIO + 512]
            idv_sb = cst16_sb[:, C_IDV:C_IDV + H]
            id16_sb = cst16_sb[0:H + 1, C_ID16:C_ID16 + H + 1]
            bkv_sb = cst32_sb[:, C_BKV:C_BKV + 1]
            bq_sb = cst32_sb[0:H, C_BQ:C_BQ + 1]
            thr_sb = cst32_sb[:, C_THR:C_THR + 2 * NKT]
            thrb_sb = cst32_sb[:, C_THRB:C_THRB + 2 * NKT]

            def xs(kt, h, c0, c1):
                base = (h * 8 + kt) * 1024
                return xk_sb[:, base + c0:base + c1]

            kvT_sb = cp.tile([128, S], DT, tag="kvT", name="kvT")  # 0:64 K^T, 64:128 V^T
            qT_sb = cp.tile([H, QR], DT, tag="qT", name="qT")      # A cols 0:512, B 512:1024
            v_sb = cp.tile([128, NKT * (H + 1)], DT, tag="v", name="v")
            # duplicates at partitions 64:127 for row-packed score pairs
            ktd_sb = cp.tile([128, S], DT, tag="ktd", name="ktd")
            qTd_sb = cp.tile([128, QR], DT, tag="qTd", name="qTd")
            vtd_sb = cp.tile([64, S], DT, tag="vtd", name="vtd")
            o_all = cp.tile([128, 8 * H], DT, tag="oall", name="oall")

            nc.vector.memset(v_sb[:], 1.0)

            # V^T -> V tile transposes, row-packed; pr 0..3 (k-tiles 0..7)
            # are emitted right after h0 to fill PE gaps while h1 streams in
            def v_transpose(pr):
                k0, k1 = 2 * pr, 2 * pr + 1
                t0 = sp.tile([128, H], DT, tag="score", name="vtr0")
                nc.tensor.transpose(
                    t0[:], vtd_sb[:, k0 * 128:(k0 + 1) * 128],
                    idv_sb[0:H, :], tile_position=(0, 0))
                t1 = sp.tile([128, H], DT, tag="score", name="vtr1")
                nc.tensor.transpose(
                    t1[:], kvT_sb[64:128, k1 * 128:(k1 + 1) * 128],
                    idv_sb[64:64 + H, :], tile_position=(64, 0))
                nc.vector.tensor_copy(
                    v_sb[:, k0 * (H + 1):k0 * (H + 1) + H], t0[:])
                nc.vector.tensor_copy(
                    v_sb[:, k1 * (H + 1):k1 * (H + 1) + H], t1[:])

            # ---- projections in two 3-bank PSUM phase scopes so the score
            # pool can allocate after phase h0 and slot-A attention overlaps
            # phase h1 (banks: h1 3 + score 4 = 7; then score 4 + av 4 = 8)
            sp = None
            for h in range(2):
                with tc.tile_pool(name=f"proj_ps{h}", bufs=1,
                                  space="PSUM") as pp:
                    kv_ps = [pp.tile([128, 512], F32, tag=f"kvps{h}{s}",
                                     name=f"kvps{h}{s}") for s in range(2)]
                    q_ps = pp.tile([H, 512], F32, tag=f"qps{h}",
                                   name=f"qps{h}")
                    # q columns: slot A = positions 0:512 (in half 0),
                    # slot B = positions 1536:2048 (in half 1)
                    qc = 0 if h == 0 else 512
                    for kt in range(8):
                        for sub in range(2):
                            nc.tensor.matmul(
                                kv_ps[sub][:],
                                wkv_sb[:, kt * 128:(kt + 1) * 128],
                                xs(kt, h, sub * 512, (sub + 1) * 512),
                                start=(kt == 0), stop=(kt == 7),
                            )
                        nc.tensor.matmul(
                            q_ps[:],
                            wq_sb[:, kt * H:(kt + 1) * H],
                            xs(kt, h, qc, qc + 512),
                            start=(kt == 0), stop=(kt == 7),
                        )
                    for sub in range(2):
                        nb = 2 * h + sub
                        nc.vector.tensor_scalar(
                            kvT_sb[:, nb * 512:(nb + 1) * 512], kv_ps[sub][:],
                            bkv_sb[:], None, add)
                        # K^T duplicate rows 64:127 (small SBUF->SBUF DMA,
                        # off the critical path)
                        nc.sync.dma_start(
                            ktd_sb[H:128, nb * 512:(nb + 1) * 512],
                            kvT_sb[0:H, nb * 512:(nb + 1) * 512])
                        nc.vector.tensor_scalar(
                            vtd_sb[:, nb * 512:(nb + 1) * 512],
                            kv_ps[sub][H:128, :], bkv_sb[H:128, :], None, add)
                    nc.vector.tensor_scalar(
                        qT_sb[:, h * 512:(h + 1) * 512], q_ps[:],
                        bq_sb[:], None, add)
                    nc.scalar.dma_start(
                        qTd_sb[H:128, h * 512:(h + 1) * 512],
                        qT_sb[:, h * 512:(h + 1) * 512])
                if h == 0:
                    sp = tc.alloc_tile_pool(name="score_ps", bufs=4,
                                            space="PSUM")
                    for pr in range(4):
                        v_transpose(pr)
            avpA = tc.alloc_tile_pool(name="avA_ps", bufs=1, space="PSUM")
            avpB = tc.alloc_tile_pool(name="avB_ps", bufs=1, space="PSUM")


            # ---- attention per slot (score pairs row-packed) ----
            for slot in range(2):
                nkt = SLOT_KT[slot]
                avp = avpA if slot == 0 else avpB
                av_e = avp.tile([H + 1, 512], F32, tag=f"avE{slot}",
                                name="avE")
                av_o = avp.tile([H + 1, 512], F32, tag=f"avO{slot}",
                                name="avO")
                # slot B: unmasked tiles (0..7) first -- their V tiles
                # exist already; v8..15 transposes are emitted here and
                # overlap the first pairs' score/exp work
                if slot == 1:
                    # end on a bias-masked pair (8..11): its exp->AV tail
                    # chain skips the diagonal pairs' extra mask multiply
                    kts = list(range(8)) + [12, 13, 14, 15, 8, 9, 10, 11]
                    for pr in range(4, NKT // 2):
                        v_transpose(pr)
                else:
                    kts = list(range(nkt))
                for ki in range(0, nkt, 2):
                    kt0, kt1 = kts[ki], kts[ki + 1]
                    s0 = sp.tile([128, 512], F32, tag="score", name="score0")
                    nc.tensor.matmul(
                        s0[:],
                        kvT_sb[0:H, kt0 * 128:(kt0 + 1) * 128],
                        qT_sb[:, slot * 512:(slot + 1) * 512],
                        start=True, stop=True, tile_position=(0, 0),
                    )
                    s1 = sp.tile([128, 512], F32, tag="score", name="score1")
                    nc.tensor.matmul(
                        s1[:],
                        ktd_sb[H:128, kt1 * 128:(kt1 + 1) * 128],
                        qTd_sb[H:128, slot * 512:(slot + 1) * 512],
                        start=True, stop=True, tile_position=(64, 0),
                    )
                    w_pair = []
                    for kt, s_ps in zip((kt0, kt1), (s0, s1)):
                        idx = slot * NKT + kt
                        w_sb = wp.tile([128, 512], DT, tag="wexp", name="wexp")
                        # bias kills constant fully-masked tiles for free
                        nc.scalar.activation(w_sb[:], s_ps[:], Exp,
                                             bias=thrb_sb[:, idx:idx + 1],
                                             scale=float(SCALE))
                        w_pair.append(w_sb)
                    wav_pair = []
                    for kt, w_sb in zip((kt0, kt1), w_pair):
                        # only diagonal tiles still need an elementwise mask
                        diag = (slot == 0 and kt < 4) or (slot == 1 and kt >= 12)
                        if not diag:
                            wav_pair.append(w_sb)
                            continue
                        idx = slot * NKT + kt
                        m_sb = wp.tile([128, 512], DT, tag="msk", name="msk")
                        nc.vector.tensor_scalar(
                            m_sb[:], qio_sb[:], thr_sb[:, idx:idx + 1], None, ge)
                        wm_sb = wp.tile([128, 512], DT, tag="wm", name="wm")
                        nc.vector.tensor_tensor(
                            wm_sb[:], w_sb[:], m_sb[:], mult)
                        wav_pair.append(wm_sb)
                    for j, (kt, w_av) in enumerate(zip((kt0, kt1), wav_pair)):
                        vs = slice(kt * (H + 1), (kt + 1) * (H + 1))
                        nc.tensor.matmul(
                            av_e[:], v_sb[0:H, vs], w_av[0:H, :],
                            start=(ki + j == 0), stop=(ki + j == nkt - 1),
                            tile_position=(0, 0),
                        )
                        nc.tensor.matmul(
                            av_o[:], v_sb[H:128, vs], w_av[H:128, :],
                            start=(ki + j == 0), stop=(ki + j == nkt - 1),
                            tile_position=(64, 0),
                        )
                # epilogue: sum AV halves (ACT copy + DVE add, fp16),
                # transpose to [128, 65], normalize in f32
                oav_sb = ep.tile([H + 1, 512], DT, tag="oav16", name="oav")
                oc_sb = ep.tile([H + 1, 512], F32, tag="oav", name="oavc")
                for j in range(4):
                    js = slice(j * 128, (j + 1) * 128)
                    nc.scalar.activation(oc_sb[:, js], av_e[:, js], Copy)
                    nc.vector.tensor_tensor(
                        oav_sb[:, js], oc_sb[:, js], av_o[:, js], add)
                for j in range(4):
                    tr_ps = sp.tile([128, H + 1], DT, tag="score", name="otr")
                    nc.tensor.transpose(
                        tr_ps[:],
                        oav_sb[:, j * 128:(j + 1) * 128],
                        id16_sb[:],
                    )
                    r_sb = ep.tile([128, 1], F32, tag="recip", name="recip")
                    nc.vector.reciprocal(r_sb[:], tr_ps[:, H:H + 1])
                    o_col = (slot * 4 + j) * H
                    nc.vector.tensor_scalar_mul(
                        o_all[:, o_col:o_col + H], tr_ps[:, 0:H], r_sb[:])
                # one [128, 256] store per slot
                (nc.sync if slot == 0 else nc.scalar).dma_start(
                    out[:, slot * 4 * H:(slot + 1) * 4 * H],
                    o_all[:, slot * 4 * H:(slot + 1) * 4 * H])

            for pool in (avpB, avpA, sp):
                pool.release()

    nc.compile()
    return nc


def _host_inputs(x, Wq, bq, Wk, bk, Wv, bv):
    """Build the 8 per-core input maps (all SBUF-layout, fp16/f32)."""
    f16 = np.float16
    Wkv = np.concatenate([Wk, Wv], axis=1)          # [D, 128]

    cst16_np = np.zeros((128, C16_N), dtype=f16)
    for kt in range(8):
        cst16_np[:, C_WKV + kt * 128:C_WKV + (kt + 1) * 128] = \
            Wkv[kt * 128:(kt + 1) * 128, :]
        cst16_np[:, C_WQ + kt * H:C_WQ + (kt + 1) * H] = \
            Wq[kt * 128:(kt + 1) * 128, :]
    cst16_np[:, C_QIO:C_QIO + CH] = np.arange(CH, dtype=f16)[None, :]
    cst16_np[:, C_IDV:C_IDV + H] = np.concatenate(
        [np.eye(H), np.eye(H)], axis=0)
    cst16_np[0:H + 1, C_ID16:C_ID16 + H + 1] = np.eye(H + 1)

    in_maps = []
    for c in range(8):
        b = c // 2
        cA, cB = c % 2, 3 - c % 2
        perm = (cA, 1 - cA, 5 - cB, cB)        # chunk order along k
        xTp = np.concatenate(
            [x[b, p * CH:(p + 1) * CH].T for p in perm], axis=1)  # [D, S]
        xTp = xTp.astype(f16)
        xk_np = np.zeros((128, 16 * 1024), dtype=f16)
        for kt in range(8):
            for h in range(2):
                s = h * 8 + kt
                xk_np[:, s * 1024:(s + 1) * 1024] = \
                    xTp[kt * 128:(kt + 1) * 128, h * 1024:(h + 1) * 1024]
        # k_global of permuted position p: perm[p//512]*512 + p%512
        pos = np.arange(S)
        kg = np.array(perm)[pos // CH] * CH + pos % CH
        thr_np = np.zeros((128, 2 * NKT), dtype=np.float32)
        p = np.arange(128)
        for slot, ck in enumerate((cA, cB)):
            for kt in range(NKT):
                thr_np[:, slot * NKT + kt] = kg[kt * 128 + p] - ck * CH
        thrb_np = np.zeros((128, 2 * NKT), dtype=np.float32)
        for slot in range(2):
            for kt in range(NKT):
                diag = (slot == 0 and kt < 4) or (slot == 1 and kt >= 12)
                if diag:
                    continue
                col = thr_np[:, slot * NKT + kt]
                if np.all(col <= 0):
                    continue          # fully visible -> bias 0
                thrb_np[:, slot * NKT + kt] = -1e5   # fully masked
        cst32_np = np.zeros((128, C32_N), dtype=np.float32)
        cst32_np[:, C_BKV] = np.concatenate([bk, bv])
        cst32_np[0:H, C_BQ] = bq
        cst32_np[:, C_THR:C_THR + 2 * NKT] = thr_np
        cst32_np[:, C_THRB:C_THRB + 2 * NKT] = thrb_np
        in_maps.append({
            "xk": xk_np, "cst16": cst16_np, "cst32": cst32_np,
        })
    return in_maps


def _gather(results, dtype):
    y = np.zeros((B, S, H), dtype=dtype)
    for c in range(8):
        b = c // 2
        cA, cB = c % 2, 3 - c % 2
        o = results[c]["out"]
        for slot, ck in enumerate((cA, cB)):
            for j in range(4):
                col = (slot * 4 + j) * H
                y[b, ck * CH + j * 128:ck * CH + (j + 1) * 128] = \
                    o[:, col:col + H]
    return y


def get_nc():
    if "nc" not in _CACHE:
        _CACHE["nc"] = _build_nc()
    return _CACHE["nc"]


def kernel(x, Wq, bq, Wk, bk, Wv, bv, _trace=False, _trace_kwargs=None):
    from concourse.bass_utils import run_bass_kernel_spmd

    x = np.asarray(x, dtype=np.float32)
    Wq, bq = np.asarray(Wq, np.float32), np.asarray(bq, np.float32)
    Wk, bk = np.asarray(Wk, np.float32), np.asarray(bk, np.float32)
    Wv, bv = np.asarray(Wv, np.float32), np.asarray(bv, np.float32)

    nc = get_nc()
    in_maps = _host_inputs(x, Wq, bq, Wk, bk, Wv, bv)
    res = run_bass_kernel_spmd(
        nc, in_maps, core_ids=list(range(8)),
        trace=_trace, **(_trace_kwargs or {}))
    _CACHE["last_result"] = res
    return _gather(res.results, x.dtype)
